# revision 1
# baseline (speedup 1.0000x reference)
"""Trainium2 Bass kernel for nn_Critic (bidirectional-LSTM critic network).

Data-parallel over the B (sequence) dimension: 8 NeuronCores x 512 sequences.
Per core, feature-major layout throughout:

  Phase T (trunk): two LayerNorm-MLP layers. Mean-centering is folded into
    the weights on the host (W @ (I - 1/64)), so LN reduces to an rsqrt of
    the per-sample sum-of-squares, computed with PE reduce/replicate matmuls.
    Timestep blocks are emitted two-ended (t, 63-t) so both LSTM directions
    could stream (phases are serialized in v1 due to ACT table sets).
  Phase L (LSTM): 64 steps, fw+bw packed on partitions [128 = 64fw|64bw, 512].
    Gates come from col-tiled matmuls; sigmoid/tanh on ACT; cell update on
    DVE; per-step head dot-products (wx, wp) via tiny matmuls, staged to DRAM.
  Phase H (head): per-row LayerNorm over T, softmax(pn @ W3) attention,
    weighted sum -> [2B] output.
"""

import os
import sys

for _p in ("/opt/trn_rl_repo",):
    if _p not in sys.path:
        sys.path.insert(0, _p)

LAST_RESULT = None  # stash of the full BassKernelResults when KERNEL_TRACE is set

import json as _json
from types import MethodType as _MethodType

import numpy as np

import concourse.bass as bass
import concourse.tile as tile
from concourse import mybir
from concourse.bass_utils import run_bass_kernel_spmd

F32 = mybir.dt.float32
F32R = mybir.dt.float32r
AF = mybir.ActivationFunctionType
ALU = mybir.AluOpType

B, T, H, OBS, ACTD = 4096, 64, 64, 128, 32
NCORES = 8
BC = B // NCORES  # 512 sequences per core
NT = T * BC  # 32768 rows per core
EPS = 1e-12

# ---------------------------------------------------------------- waitfix --
# This walrus build rejects instructions carrying more than one sync-wait
# command. The Tile kernel-tail drain (and barriers) routinely carry more.
# Patch the serialized BIR: move excess waits onto inserted NoOp carriers.
_MAX_WAITS = 1


def _patch_bir(bir):
    n = [0]

    def fresh():
        n[0] += 1
        return f"I-waitfix-{n[0]}"

    for fn in bir.get("functions", []):
        for bb in fn.get("blocks", []):
            out = []
            for inst in bb.get("instructions", []):
                si = inst.get("sync_info") or {}
                waits = si.get("on_wait") or []
                if len(waits) > _MAX_WAITS:
                    extra = waits[: len(waits) - _MAX_WAITS]
                    keep = waits[len(waits) - _MAX_WAITS :]
                    for i in range(0, len(extra), _MAX_WAITS):
                        out.append(
                            {
                                "name": fresh(),
                                "opcode": "NoOp",
                                "engine": inst["engine"],
                                "ins": [],
                                "outs": [],
                                "sync_info": {
                                    "on_wait": extra[i : i + _MAX_WAITS],
                                    "on_update": [],
                                },
                            }
                        )
                    si = dict(si)
                    si["on_wait"] = keep
                    inst = dict(inst)
                    inst["sync_info"] = si
                out.append(inst)
            bb["instructions"] = out
    return bir


def _install_waitfix(nc):
    orig = nc.to_json_bytes

    def patched(self):
        return _json.dumps(_patch_bir(_json.loads(orig()))).encode()

    nc.to_json_bytes = _MethodType(patched, nc)
    return nc


# ----------------------------------------------------------- host weights --


def _prep_consts(p):
    """Fold reference weights into device layouts. p: dict of np arrays."""
    f32 = lambda x: np.ascontiguousarray(x, dtype=np.float32)
    C = np.eye(64, dtype=np.float64) - 1.0 / 64.0

    c = {}
    c["W1c"] = f32(p["W1"].astype(np.float64) @ C)  # [128, 64]
    c["b1c"] = f32(p["b1"].astype(np.float64) @ C)  # [64]
    c["W2c"] = f32(p["W2"].astype(np.float64) @ C)  # [96, 64]
    c["b2c"] = f32(p["b2"].astype(np.float64) @ C)  # [64]
    c["g1be1"] = f32(np.stack([np.tile(p["g1"], 2), np.tile(p["be1"], 2)], 1))
    c["g2be2"] = f32(np.stack([np.tile(p["g2"], 2), np.tile(p["be2"], 2)], 1))

    # Gate weights: lhsT rows = xh rows = [h (64); x (64)]; reference W rows =
    # [x (64); h (64)].  Columns i|j|f|o stay in place.
    c["Wgf"] = f32(np.concatenate([p["Wf"][64:128], p["Wf"][0:64]], 0))
    c["Wgb"] = f32(np.concatenate([p["Wb"][64:128], p["Wb"][0:64]], 0))
    gb = np.zeros((128, 4), np.float64)
    for gi, sl in enumerate((slice(0, 64), slice(64, 128), slice(128, 192), slice(192, 256))):
        gb[0:64, gi] = p["bf"][sl]
        gb[64:128, gi] = p["bb"][sl]
    gb[:, 2] += 1.0  # forget_bias
    c["gbias"] = f32(gb)  # cols: i, j, f, o

    c["whead"] = f32(np.stack([p["wx"], p["wp"]], 1))  # [64, 2]

    # fp32r matmuls need tile_position (0,0): zero-padded lhsT halves
    z64 = np.zeros((128, 64), np.float64)
    c["W1pa"] = f32(np.concatenate([c["W1c"], z64[:, :64] * 0], 1)[:, :128] * 1)
    c["W1pa"] = f32(np.concatenate([c["W1c"].astype(np.float64), np.zeros((128, 64))], 1))
    c["W1pb"] = f32(np.concatenate([np.zeros((128, 64)), c["W1c"].astype(np.float64)], 1))
    c["W2pa"] = f32(np.concatenate([c["W2c"].astype(np.float64), np.zeros((96, 64))], 1))
    c["W2pb"] = f32(np.concatenate([np.zeros((96, 64)), c["W2c"].astype(np.float64)], 1))
    for gi in range(4):
        gf = c["Wgf"][:, gi * 64 : (gi + 1) * 64].astype(np.float64)
        gb_ = c["Wgb"][:, gi * 64 : (gi + 1) * 64].astype(np.float64)
        c[f"Wgp{gi}0"] = f32(np.concatenate([gf, np.zeros((128, 64))], 1))
        c[f"Wgp{gi}1"] = f32(np.concatenate([np.zeros((128, 64)), gb_], 1))
    c["CT"] = f32(np.eye(64) - 1.0 / 64.0)
    c["ones_red"] = f32(
        np.block(
            [[np.ones((64, 1)), np.zeros((64, 1))], [np.zeros((64, 1)), np.ones((64, 1))]]
        )
    )  # [128, 2]
    c["ones_rep"] = f32(
        np.block(
            [[np.ones((1, 64)), np.zeros((1, 64))], [np.zeros((1, 64)), np.ones((1, 64))]]
        )
    )  # [2, 128]
    c["ones64"] = f32(np.ones((64, 1)))
    c["ones_1_64"] = f32(np.ones((1, 64)))

    # Head params; bw direction is time-reversed relative to our bw scan
    # state order, handled by reversing the per-t parameters.
    c["W3f"] = f32(p["W3"])
    c["W3b"] = f32(p["W3"][::-1, ::-1])
    c["b3f"] = f32(p["b3"].reshape(64, 1))
    c["b3b"] = f32(p["b3"][::-1].reshape(64, 1))
    c["gpbepf"] = f32(np.stack([p["gp"], p["bep"]], 1))  # [64, 2]
    c["gpbepb"] = f32(np.stack([p["gp"][::-1], p["bep"][::-1]], 1))
    c["bx"] = float(np.asarray(p["bx"]))
    return c


# ------------------------------------------------------------ bass program --


# matmul lhsT weights: declared float32r so the PE runs 1 cycle/row
_MM_WEIGHTS = (
    "W1c", "W2c", "Wgf", "Wgb", "whead", "CT", "ones_red", "ones_rep",
    "ones64", "ones_1_64", "W3f", "W3b",
    "W1pa", "W1pb", "W2pa", "W2pb",
    "Wgp00", "Wgp01", "Wgp10", "Wgp11", "Wgp20", "Wgp21", "Wgp30", "Wgp31",
)


def _build(consts):
    nc = bass.Bass()
    obsT = nc.declare_dram_parameter("obsT", [128, NT], F32R, isOutput=False)
    actT = nc.declare_dram_parameter("actT", [32, NT], F32R, isOutput=False)

    cin = {}
    for name in (
        "W1c", "b1c", "W2c", "b2c", "g1be1", "g2be2", "Wgf", "Wgb", "gbias",
        "whead", "CT", "ones_red", "ones_rep", "ones64", "ones_1_64",
        "W3f", "W3b", "b3f", "b3b", "gpbepf", "gpbepb",
        "W1pa", "W1pb", "W2pa", "W2pb",
        "Wgp00", "Wgp01", "Wgp10", "Wgp11", "Wgp20", "Wgp21", "Wgp30", "Wgp31",
    ):
        a = consts[name]
        shp = list(a.shape) if a.ndim == 2 else [a.shape[0], 1]
        dt_ = F32R if name in _MM_WEIGHTS else F32
        cin[name] = nc.declare_dram_parameter(name, shp, dt_, isOutput=False)

    ov = nc.declare_dram_parameter("ov", [2, BC], F32, isOutput=True)
    xs_dram = nc.dram_tensor("xs_stash", [4, NT], F32R)

    b1_nz = bool(np.any(consts["b1c"]))
    b2_nz = bool(np.any(consts["b2c"]))

    def mmr(out, lhsT, rhs, **kw):
        assert lhsT.dtype == F32R and rhs.dtype == F32R, (lhsT.dtype, rhs.dtype)
        nc.tensor.matmul(out, lhsT, rhs, **kw)

    with tile.TileContext(nc) as tc:
        with (
            tc.tile_pool(name="singles", bufs=1) as sing,
            tc.tile_pool(name="obs_p", bufs=4) as obs_p,
            tc.tile_pool(name="act_p", bufs=4) as act_p,
            tc.tile_pool(name="big", bufs=2) as big,
            tc.tile_pool(name="small", bufs=4) as small,
        ):
            # ---- load constants ----
            ct = {}
            for name, dram in cin.items():
                a = consts[name]
                shp = list(a.shape) if a.ndim == 2 else [a.shape[0], 1]
                ct[name] = sing.tile(shp, F32R if name in _MM_WEIGHTS else F32, name=f"ct_{name}", tag=f"ct_{name}")
                nc.sync.dma_start(out=ct[name], in_=dram[:, :])

            # fp32r matmuls require tile_position == (0, 0): zero-padded lhsT
            # halves (host-prepared consts) + psum accumulate.
            W1p = [ct["W1pa"], ct["W1pb"]]
            W2p = [ct["W2pa"], ct["W2pb"]]
            Wgp = {(gi, di): ct[f"Wgp{gi}{di}"] for gi in range(4) for di in range(2)}

            X2 = sing.tile([128, 32 * BC], F32R)  # x2, two-ended t-pair packing
            epst = sing.tile([128, 1], F32)
            nc.vector.memset(epst, EPS)

            # b1c/b2c as [128,1] per-partition (2-stacked) if needed
            if b1_nz or b2_nz:
                bstk = sing.tile([128, 2], F32)
                # col 0 = [b1c;b1c], col 1 = [b2c;b2c] via DMA broadcast: the
                # host passes b1c/b2c as [64,1]; replicate by two DMAs each.
                for col, nm in ((0, "b1c"), (1, "b2c")):
                    nc.sync.dma_start(out=bstk[0:64, col : col + 1], in_=cin[nm][:, :])
                    nc.sync.dma_start(out=bstk[64:128, col : col + 1], in_=cin[nm][:, :])

            # ================= Phase T: trunk =================
            trunk_ps = tc.tile_pool(name="trunk_ps", bufs=2, space="PSUM")
            with trunk_ps as ps_v, tc.tile_pool(name="trunk_ps2", bufs=2, space="PSUM") as ps_s, tc.tile_pool(name="trunk_ps3", bufs=2, space="PSUM") as ps_r:
              for j in range(32):
                  ta, tb = j, 63 - j
                  oa = obs_p.tile([128, BC], F32R, tag="obs")
                  nc.sync.dma_start(out=oa, in_=obsT[:, ta * BC : (ta + 1) * BC])
                  ob = obs_p.tile([128, BC], F32R, tag="obs")
                  nc.sync.dma_start(out=ob, in_=obsT[:, tb * BC : (tb + 1) * BC])

                  # ---- layer 1 ----
                  pv = ps_v.tile([128, BC], F32, tag="pv")
                  mmr(pv, W1p[0], oa, start=True, stop=False)
                  mmr(pv, W1p[1], ob, start=False, stop=True)
                  vsq = big.tile([128, BC], F32R, tag="vsq")
                  v_s = big.tile([128, BC], F32, tag="v_s")
                  if b1_nz:
                      nc.scalar.activation(vsq, pv, AF.Square, bias=bstk[:, 0:1])
                      nc.scalar.activation(v_s, pv, AF.Identity, bias=bstk[:, 0:1])
                  else:
                      nc.scalar.activation(vsq, pv, AF.Square)
                      nc.vector.tensor_copy(v_s, pv)
                  pss = ps_s.tile([2, BC], F32, tag="pss")
                  mmr(pss, ct["ones_red"], vsq, start=True, stop=True)
                  stdv = small.tile([2, BC], F32, tag="stdv")
                  nc.scalar.activation(stdv, pss, AF.Sqrt, bias=epst[0:2, 0:1], scale=1.0 / 64.0)
                  rstd = small.tile([2, BC], F32R, tag="rstd")
                  with nc.allow_low_precision(reason="fp32r rstd"):
                      nc.vector.reciprocal(rstd, stdv)
                  prep = ps_r.tile([128, BC], F32, tag="prep")
                  mmr(prep, ct["ones_rep"], rstd, start=True, stop=True)
                  xn = big.tile([128, BC], F32, tag="xn")
                  nc.vector.tensor_mul(xn, v_s, prep)
                  xa = big.tile([96, BC], F32R, tag="xa")
                  xb = big.tile([96, BC], F32R, tag="xb")
                  nc.scalar.activation(xa[0:64, :], xn[0:64, :], AF.Relu,
                                       bias=ct["g1be1"][0:64, 1:2],
                                       scale=ct["g1be1"][0:64, 0:1])
                  nc.scalar.activation(xb[0:64, :], xn[64:128, :], AF.Relu,
                                       bias=ct["g1be1"][64:128, 1:2],
                                       scale=ct["g1be1"][64:128, 0:1])
                  nc.sync.dma_start(out=xa[64:96, :], in_=actT[:, ta * BC : (ta + 1) * BC])
                  nc.sync.dma_start(out=xb[64:96, :], in_=actT[:, tb * BC : (tb + 1) * BC])

                  # ---- layer 2 ----
                  pv2 = ps_v.tile([128, BC], F32, tag="pv")
                  mmr(pv2, W2p[0], xa[0:96, :], start=True, stop=False)
                  mmr(pv2, W2p[1], xb[0:96, :], start=False, stop=True)
                  vsq2 = big.tile([128, BC], F32R, tag="vsq")
                  v_s2 = big.tile([128, BC], F32, tag="v_s")
                  if b2_nz:
                      nc.scalar.activation(vsq2, pv2, AF.Square, bias=bstk[:, 1:2])
                      nc.scalar.activation(v_s2, pv2, AF.Identity, bias=bstk[:, 1:2])
                  else:
                      nc.scalar.activation(vsq2, pv2, AF.Square)
                      nc.vector.tensor_copy(v_s2, pv2)
                  pss2 = ps_s.tile([2, BC], F32, tag="pss")
                  mmr(pss2, ct["ones_red"], vsq2, start=True, stop=True)
                  stdv2 = small.tile([2, BC], F32, tag="stdv")
                  nc.scalar.activation(stdv2, pss2, AF.Sqrt, bias=epst[0:2, 0:1], scale=1.0 / 64.0)
                  rstd2 = small.tile([2, BC], F32R, tag="rstd")
                  with nc.allow_low_precision(reason="fp32r rstd"):
                      nc.vector.reciprocal(rstd2, stdv2)
                  prep2 = ps_r.tile([128, BC], F32, tag="prep")
                  mmr(prep2, ct["ones_rep"], rstd2, start=True, stop=True)
                  xn2 = big.tile([128, BC], F32, tag="xn")
                  nc.vector.tensor_mul(xn2, v_s2, prep2)
                  # write both t-blocks of x2 into X2 block j in one op
                  nc.scalar.activation(X2[:, j * BC : (j + 1) * BC], xn2, AF.Relu,
                                       bias=ct["g2be2"][:, 1:2],
                                       scale=ct["g2be2"][:, 0:1])

            tc.strict_bb_all_engine_barrier()

            # ================= Phase L: LSTM =================
            xh_f = sing.tile([128, BC], F32R)  # rows 0:64 h_fw, 64:128 x_fw
            xh_b = sing.tile([128, BC], F32R)
            cst = sing.tile([128, BC], F32)  # cell state [c_fw; c_bw]
            zini = sing.tile([64, BC], F32)
            nc.vector.memset(zini, 0.0)
            nc.vector.tensor_copy(xh_f[0:64, :], zini)
            nc.vector.tensor_copy(xh_b[0:64, :], zini)
            nc.vector.memset(cst, 0.0)

            lstm_ps_cm = tc.tile_pool(name="lstm_ps", bufs=1, space="PSUM")
            lstm_ps = lstm_ps_cm.__enter__()
            pg_i = lstm_ps.tile([128, BC], F32, tag="pg_i")
            pg_j = lstm_ps.tile([128, BC], F32, tag="pg_j")
            pg_f = lstm_ps.tile([128, BC], F32, tag="pg_f")
            pg_o = lstm_ps.tile([128, BC], F32, tag="pg_o")
            ph_f = [lstm_ps.tile([2, 2 * BC], F32, name=f"ph_f{i}", tag=f"ph_f{i}") for i in range(1)]
            ph_b = [lstm_ps.tile([2, 2 * BC], F32, name=f"ph_b{i}", tag=f"ph_b{i}") for i in range(1)]
            stg_f = [sing.tile([2, 2 * BC], F32R, name=f"stg_f{i}", tag=f"stg_f{i}") for i in range(2)]
            stg_b = [sing.tile([2, 2 * BC], F32R, name=f"stg_b{i}", tag=f"stg_b{i}") for i in range(2)]

            def xsrc(t_needed):
                if t_needed < 32:
                    return X2[0:64, t_needed * BC : (t_needed + 1) * BC]
                jj = 63 - t_needed
                return X2[64:128, jj * BC : (jj + 1) * BC]

            gates = ((pg_i, 0, AF.Sigmoid), (pg_j, 1, AF.Tanh),
                     (pg_f, 2, AF.Sigmoid), (pg_o, 3, AF.Sigmoid))

            for t in range(T):
                nc.vector.tensor_copy(xh_f[64:128, :], xsrc(t))
                nc.vector.tensor_copy(xh_b[64:128, :], xsrc(63 - t))
                for pg, gi, _fn in gates:
                    mmr(pg, Wgp[(gi, 0)], xh_f, start=True, stop=False)
                    mmr(pg, Wgp[(gi, 1)], xh_b, start=False, stop=True)
                sI = big.tile([128, BC], F32, tag="sI")
                tJ = big.tile([128, BC], F32, tag="tJ")
                sF = big.tile([128, BC], F32, tag="sF")
                sO = big.tile([128, BC], F32, tag="sO")
                for (pg, gi, fn), dst in zip(gates, (sI, tJ, sF, sO)):
                    nc.scalar.activation(dst, pg, fn, bias=ct["gbias"][:, gi : gi + 1])
                u = big.tile([128, BC], F32, tag="u")
                nc.vector.tensor_mul(u, sI, tJ)
                cf = big.tile([128, BC], F32, tag="cf")
                nc.vector.tensor_mul(cf, cst, sF)
                nc.vector.tensor_add(cst, cf, u)
                tcl = big.tile([128, BC], F32, tag="tc")
                nc.scalar.activation(tcl, cst, AF.Tanh)
                nc.vector.tensor_mul(xh_f[0:64, :], tcl[0:64, :], sO[0:64, :])
                nc.vector.tensor_mul(xh_b[0:64, :], tcl[64:128, :], sO[64:128, :])

                # head dots: [xs; ps] rows for this step
                ph = ph_f[0]
                pb = ph_b[0]
                col = (t % 2) * BC
                mmr(ph[0:2, col : col + BC], ct["whead"], xh_f[0:64, :],
                                 start=True, stop=True, tile_position=(0, 0))
                mmr(pb[0:2, col : col + BC], ct["whead"], xh_b[0:64, :],
                                 start=True, stop=True, tile_position=(0, 0))
                if t % 2 == 1:
                    sf = stg_f[(t // 2) % 2]
                    sb = stg_b[(t // 2) % 2]
                    nc.vector.tensor_copy(sf, ph)
                    nc.vector.tensor_copy(sb, pb)
                    dcol = (t - 1) * BC
                    nc.sync.dma_start(out=xs_dram[0:2, dcol : dcol + 2 * BC], in_=sf)
                    nc.sync.dma_start(out=xs_dram[2:4, dcol : dcol + 2 * BC], in_=sb)

            lstm_ps_cm.__exit__(None, None, None)
            tc.strict_bb_all_engine_barrier()

            # ================= Phase H: head =================
            head_ps_cm = tc.tile_pool(name="head_ps", bufs=1, space="PSUM")
            ps_h = head_ps_cm.__enter__()
            pn_d = []
            xs_d = []
            for d, (w3, b3, gpb) in enumerate(
                (("W3f", "b3f", "gpbepf"), ("W3b", "b3b", "gpbepb"))
            ):
                xsT = big.tile([64, BC], F32R, tag="u")
                psT = big.tile([64, BC], F32R, tag="cf")
                nc.sync.dma_start(
                    out=xsT, in_=xs_dram[2 * d : 2 * d + 1, :].rearrange("o (t b) -> (o t) b", b=BC)
                )
                nc.sync.dma_start(
                    out=psT, in_=xs_dram[2 * d + 1 : 2 * d + 2, :].rearrange("o (t b) -> (o t) b", b=BC)
                )
                pc = ps_h.tile([64, BC], F32, tag="hpc")
                mmr(pc, ct["CT"], psT, start=True, stop=True)
                hsq = big.tile([64, BC], F32R, tag="vsq")
                hcs = big.tile([64, BC], F32, tag="v_s")
                nc.scalar.activation(hsq, pc, AF.Square)
                nc.vector.tensor_copy(hcs, pc)
                hss = ps_h.tile([1, BC], F32, tag="hss")
                mmr(hss, ct["ones64"], hsq, start=True, stop=True)
                hstd = small.tile([1, BC], F32, tag="stdv")
                nc.scalar.activation(hstd, hss, AF.Sqrt, bias=epst[0:1, 0:1], scale=1.0 / 64.0)
                hrst = small.tile([1, BC], F32R, tag="rstd")
                with nc.allow_low_precision(reason="fp32r rstd"):
                    nc.vector.reciprocal(hrst, hstd)
                hrep = ps_h.tile([64, BC], F32, tag="hrep")
                mmr(hrep, ct["ones_1_64"], hrst, start=True, stop=True)
                ht1 = big.tile([64, BC], F32, tag="xn")
                nc.vector.tensor_mul(ht1, hcs, hrep)
                pn = big.tile([64, BC], F32R, tag="tc")
                nc.scalar.activation(pn, ht1, AF.Relu, bias=ct[gpb][:, 1:2],
                                     scale=ct[gpb][:, 0:1])
                pn_d.append((pn, w3, b3))
                xs_d.append(xsT)

            ovs0 = sing.tile([1, BC], F32)
            ovs1 = sing.tile([1, BC], F32)
            for d, ((pn, w3, b3), xsT) in enumerate(zip(pn_d, xs_d)):
                pl = ps_h.tile([64, BC], F32, tag="hpl")
                mmr(pl, ct[w3], pn, start=True, stop=True)
                he = big.tile([64, BC], F32R, tag="sI")
                nc.scalar.activation(he, pl, AF.Exp, bias=ct[b3][:, 0:1])
                hse = ps_h.tile([1, BC], F32, tag="hse")
                mmr(hse, ct["ones64"], he, start=True, stop=True)
                hrs = small.tile([1, BC], F32, tag="rstd")
                nc.vector.reciprocal(hrs, hse)
                hex = big.tile([64, BC], F32R, tag="tJ")
                nc.vector.tensor_mul(hex, he, xsT)
                hnum = ps_h.tile([1, BC], F32, tag="hnum")
                mmr(hnum, ct["ones64"], hex, start=True, stop=True)
                hov = small.tile([1, BC], F32, tag="stdv")
                nc.vector.tensor_mul(hov, hrs, hnum)
                nc.vector.tensor_scalar(
                    ovs0 if d == 0 else ovs1, hov, float(consts["bx"]), None, ALU.add
                )
            nc.sync.dma_start(out=ov[0:1, :], in_=ovs0)
            nc.sync.dma_start(out=ov[1:2, :], in_=ovs1)
            head_ps_cm.__exit__(None, None, None)

    return nc


_CACHE = {}


def kernel(**inputs):
    obs = np.ascontiguousarray(inputs["obs"], dtype=np.float32)
    action = np.ascontiguousarray(inputs["action"], dtype=np.float32)
    consts = _prep_consts(inputs)

    key = "nc"
    if key not in _CACHE:
        _CACHE[key] = _install_waitfix(_build(consts))
    nc = _CACHE[key]

    const_feed = {}
    for name, a in consts.items():
        if name == "bx":
            continue
        const_feed[name] = a if a.ndim == 2 else a.reshape(-1, 1)

    in_maps = []
    for c in range(NCORES):
        sl = slice(c * BC * T, (c + 1) * BC * T)
        obsT = np.ascontiguousarray(
            obs[sl].reshape(BC, T, OBS).transpose(2, 1, 0).reshape(OBS, NT)
        )
        actT = np.ascontiguousarray(
            action[sl].reshape(BC, T, ACTD).transpose(2, 1, 0).reshape(ACTD, NT)
        )
        m = {"obsT": obsT, "actT": actT}
        m.update(const_feed)
        in_maps.append(m)

    trace = bool(os.environ.get("KERNEL_TRACE"))
    full = run_bass_kernel_spmd(nc, in_maps, list(range(NCORES)), trace=trace)
    if trace:
        global LAST_RESULT
        LAST_RESULT = full
    res = full.results

    out = np.empty(2 * B, dtype=np.float32)
    for c in range(NCORES):
        ovc = res[c]["ov"]
        out[c * BC : (c + 1) * BC] = ovc[0]
        out[B + c * BC : B + (c + 1) * BC] = ovc[1]
    return out

